# revision 41
# baseline (speedup 1.0000x reference)
"""HeteroClassifier GNN kernel for 8 TRN2 NeuronCores (Bass/Tile).

Sharding: L1 edges by dst node-range (owner core aggregates its nodes);
L2 edges by src node-range (gather tables stay core-local); per-core
[B,2] partial outputs are summed to unshard. Host does structure-only
prep (degree counts, edge grouping, padded window/stream layouts, index
maps). All value compute runs on the NeuronCores.

Gather mechanisms (HW-validated):
- L1 and L2 edge gathers: gpsimd ap_gather from SBUF-replicated bf16
  tables (feat in 28672-node chunks for int16 indices; g per relation);
  each 16-partition group processes its own edge stream with masked
  coefficient tables, then windowed segment reduction. Streams are
  double-buffered so gathers overlap the DVE mask/reduce work, and
  reductions land directly in natural (lane, window) order (no realign).
- Final graph realign: per-column indirect DMA (32 columns).
"""

import numpy as np

import concourse.bass as bass
import concourse.bacc as bacc
import concourse.mybir as mybir
import concourse.tile as tile
from concourse import bass_utils

LAST_EXEC_NS = -1
LAST_TRACE = None
N = 200000
R = 4
E = 1000000
B = 1024
NCORES = 8
P = 128
G16 = 16          # partitions per gpsimd group
NG = P // G16     # 8 groups


def _sizes():
    nb = N // NCORES
    nwin1 = (nb + P - 1) // P
    nwin2 = (B + P - 1) // P
    return nb, nwin1, nwin2


def _pack_apg(lane, win, row, coefs, nwin, seg_len):
    """Pack an edge stream for ap_gather with group-shared indices.

    lane/win: target slot (lane in [0,P), win in [0,nwin)); row: int16
    table row to gather; coef: f32 coefficient (applied at target lane).
    seg_len[w]: shared per-window segment length (max over groups/cores).
    Returns (idx_tile [P, S//16] i16, coeff [P, S] f32) with
    S = 16*ceil(sum(seg_len)/16)."""
    S0 = int(np.sum(seg_len))
    S = ((S0 + 15) // 16) * 16
    sbase = np.concatenate([[0], np.cumsum(seg_len)]).astype(np.int64)
    g = lane // G16
    c16 = lane % G16
    # position within (group, window)
    order = np.lexsort((np.arange(len(lane)), win, g))
    gs, ws = g[order], win[order]
    # j_in_seg: running index within each (g, w) bucket
    key = gs * nwin + ws
    ksorted = key  # already sorted by (g, w)
    starts = np.searchsorted(ksorted, np.arange(NG * nwin))
    j_in = np.arange(len(ksorted)) - starts[ksorted]
    jpos = sbase[ws] + j_in
    import ml_dtypes
    idx_tile = np.zeros((P, S // 16), dtype=np.int16)
    rs, ls_ = row[order], c16[order]
    idx_tile[gs * G16 + (jpos % 16), jpos // 16] = rs
    outs = []
    for coef in coefs:
        coeff = np.zeros((P, S), dtype=ml_dtypes.bfloat16)
        coeff[gs * G16 + ls_, jpos] = coef[order]
        outs.append(coeff)
    return idx_tile, outs, S


def _prep(feat, src, dst, ew, graph_ids):
    nb, nwin1, nwin2 = _sizes()
    src = np.asarray(src); dst = np.asarray(dst)
    ew = np.asarray(ew); gid = np.asarray(graph_ids)

    od = np.stack([np.bincount(src[r], minlength=N) for r in range(R)])
    idg = np.stack([np.bincount(dst[r], minlength=N) for r in range(R)])
    cnt = np.bincount(gid, minlength=B)
    ods = (1.0 / np.sqrt(np.clip(od, 1, None))).astype(np.float32)
    ids = (1.0 / np.sqrt(np.clip(idg, 1, None))).astype(np.float32)
    qn = (ids / np.clip(cnt, 1, None)[gid][None, :]).astype(np.float32)

    meta = {"L1": [], "L2": [], "X": {}}
    per_core = [dict() for _ in range(NCORES)]

    # ---- L1: dst-sharded ap_gather streams; gather table = feat in
    # 32768-row bf16 chunks (int16 index limit). Reduce output lands in
    # natural (lane, win) order, so no realign stage is needed.
    CH = 28672
    NCH = (N + CH - 1) // CH
    meta["CH"], meta["NCH"] = CH, NCH
    for r in range(R):
        core_of = dst[r] // nb
        chunk_of = src[r] // CH
        rel_meta = []
        for ch in range(NCH):
            seg = np.ones(nwin1, dtype=np.int64)
            dat = []
            for c in range(NCORES):
                m = (core_of == c) & (chunk_of == ch)
                dl = dst[r][m] - c * nb
                lane = (dl % P).astype(np.int64)
                win = (dl // P).astype(np.int64)
                row = (src[r][m] - ch * CH).astype(np.int64)
                dat.append((lane, win, row, ew[r][m],
                            ods[r][src[r][m]].astype(np.float32)))
                cnts = np.bincount((lane // G16) * nwin1 + win,
                                   minlength=NG * nwin1)
                seg = np.maximum(seg, cnts.reshape(NG, nwin1).max(axis=0))
            # round to multiples of 4 so equal-length windows merge into
            # few strided reduce instructions
            seg = ((seg + 3) // 4) * 4
            S = int(((seg.sum() + 15) // 16) * 16)
            rel_meta.append({"seg": seg.tolist(), "S": S})
            for c in range(NCORES):
                lane, win, row, vew, vos = dat[c]
                idx_t, (cew, cos), S2 = _pack_apg(
                    lane, win, row, [vew, vos], nwin1, seg)
                assert S2 == S
                per_core[c][f"l1idx_{r}_{ch}"] = idx_t
                per_core[c][f"l1ew_{r}_{ch}"] = cew
                per_core[c][f"l1os_{r}_{ch}"] = cos
        meta["L1"].append(rel_meta)
        for c in range(NCORES):
            lo = c * nb
            on = np.zeros(nwin1 * P, dtype=np.float32)
            on[:nb] = ids[r, lo:lo + nb]
            per_core[c][f"idsl_{r}"] = on.reshape(nwin1, P).T.copy()
            on2 = np.zeros(nwin1 * P, dtype=np.float32)
            on2[:nb] = ods[r, lo:lo + nb]
            per_core[c][f"odsl_{r}"] = on2.reshape(nwin1, P).T.copy()

    # ---- L2: src-sharded, ap_gather streams grouped by graph rank
    # shared graph ranking per (relation): by per-core counts is fine but
    # ranks must be shared across cores? gpos handles per-core; use global
    # per-relation ranking by total count so window segments are shared.
    for r in range(R):
        gcnt = np.bincount(gid[dst[r]], minlength=B)
        gorder = np.argsort(-gcnt, kind="stable")
        grp = np.empty(B, dtype=np.int64)
        grp[gorder] = np.arange(B)
        core_of = src[r] // nb
        # shared segment lengths: max over cores/groups per window
        seg = np.zeros(nwin2, dtype=np.int64)
        percore_dat = []
        for c in range(NCORES):
            m = core_of == c
            d = dst[r][m]
            rk = grp[gid[d]]
            lane = rk % P
            win = rk // P
            n = src[r][m] - c * nb
            row = (n % P) * nwin1 + n // P
            coefv = qn[r][d]
            percore_dat.append((lane, win, row, coefv))
            cnts = np.bincount((lane // G16) * nwin2 + win,
                               minlength=NG * nwin2)
            seg = np.maximum(seg, cnts.reshape(NG, nwin2).max(axis=0))
        S2 = int(((seg.sum() + 15) // 16) * 16)
        meta["L2"].append({"seg": seg.tolist(), "S": S2})
        for c in range(NCORES):
            lane, win, row, coefv = percore_dat[c]
            idx_t, (coeff,), S = _pack_apg(lane, win, row.astype(np.int64),
                                           [coefv], nwin2, seg)
            assert S == S2
            per_core[c][f"l2idx_{r}"] = idx_t
            per_core[c][f"l2coef_{r}"] = coeff
        # gpos: natural graph b=(k*P+p) -> rank row (lane-major) in prank
        for c in range(NCORES):
            pos = np.full(nwin2 * P, nwin2 * P, dtype=np.int32)
            bb = np.arange(B)
            rk = grp[bb]
            pos[bb] = ((rk % P) * nwin2 + rk // P).astype(np.int32)
            per_core[c][f"gpos_{r}"] = pos.reshape(nwin2, P).T.copy()

    return per_core, meta


def _build_program(meta):
    nb, nwin1, nwin2 = _sizes()
    nc = bacc.Bacc("TRN2", target_bir_lowering=False, debug=False,
                   num_devices=NCORES)
    f32, i32 = mybir.dt.float32, mybir.dt.int32
    bf16, i16 = mybir.dt.bfloat16, mybir.dt.int16
    AL = mybir.AluOpType

    feat = nc.dram_tensor("feat", [N, 2], f32, kind="ExternalInput").ap()
    w1p = nc.dram_tensor("w1p", [P, 16 * 2 * R], f32, kind="ExternalInput").ap()
    b1b = nc.dram_tensor("b1b", [P, R * 16], f32, kind="ExternalInput").ap()
    W2 = nc.dram_tensor("W2", [R, 16, 16], f32, kind="ExternalInput").ap()
    b2b = nc.dram_tensor("b2b", [P, R * 16], f32, kind="ExternalInput").ap()
    Wc = nc.dram_tensor("Wc", [16, 2], f32, kind="ExternalInput").ap()
    bc = nc.dram_tensor("bc", [2], f32, kind="ExternalInput").ap()
    CH, NCH = meta["CH"], meta["NCH"]
    ins = {}
    for r in range(R):
        S2 = meta["L2"][r]["S"]
        for nm, shp, dt in (
            (f"odsl_{r}", [P, nwin1], f32), (f"idsl_{r}", [P, nwin1], f32),
            (f"l2idx_{r}", [P, S2 // 16], i16), (f"l2coef_{r}", [P, S2], bf16),
            (f"gpos_{r}", [P, nwin2], i32),
        ):
            ins[nm] = nc.dram_tensor(nm, shp, dt, kind="ExternalInput").ap()
        for ch in range(NCH):
            S1 = meta["L1"][r][ch]["S"]
            for nm, shp, dt in (
                (f"l1idx_{r}_{ch}", [P, S1 // 16], i16),
                (f"l1ew_{r}_{ch}", [P, S1], bf16),
                (f"l1os_{r}_{ch}", [P, S1], bf16),
            ):
                ins[nm] = nc.dram_tensor(nm, shp, dt, kind="ExternalInput").ap()
    featB = nc.dram_tensor("featB", [N * 2], bf16, kind="Internal").ap()
    gtabB = [nc.dram_tensor(f"gB_{r}", [nwin1 * P * 2], bf16, kind="Internal").ap()
             for r in range(R)]
    prank = [nc.dram_tensor(f"prank_{r}", [(nwin2 + 1) * P, 2], f32, kind="Internal").ap()
             for r in range(R)]
    out_part = nc.dram_tensor("out_part", [B, 2], f32, kind="ExternalOutput").ap()
    bias_out = nc.dram_tensor("bias_out", [1, 2], f32, kind="ExternalOutput").ap()

    NE1 = nwin1 * P  # 25088 table rows

    def reduce_windows(ga, out_t, ls, nwin):
        col = 0
        k = 0
        while k < nwin:
            k2 = k
            while k2 < nwin and ls[k2] == ls[k]:
                k2 += 1
            lk, nk = ls[k], k2 - k
            seg = ga[:, col:col + nk * lk, :].rearrange(
                "p (n l) c -> p n c l", l=lk)
            nc.vector.tensor_reduce(out=out_t[:, k:k2, :], in_=seg,
                                    op=AL.add, axis=mybir.AxisListType.X)
            col += nk * lk
            k = k2

    with tile.TileContext(nc) as tc:
        with (tc.tile_pool(name="glob", bufs=1) as gpool,
              tc.tile_pool(name="psum", bufs=2, space="PSUM") as psum):
            zt = gpool.tile([P, 2], f32, name="zt")
            nc.vector.memset(zt[:], 0.0)
            for r in range(R):
                nc.sync.dma_start(out=prank[r][nwin2 * P:, :], in_=zt[:])

            pr_ts = []

            # ---- phase 1: L1 via chunked ap_gather from bf16 feat
            # tables; masked coefficients; reduce lands in natural order
            x_t = gpool.tile([P, nwin1, 2 * R], f32, name="x_t")
            with tc.tile_pool(name="p0", bufs=1) as p0:
                FP = (N * 2) // P
                ft = p0.tile([P, FP], f32, name="ft")
                nc.sync.dma_start(
                    out=ft[:],
                    in_=feat.rearrange("n c -> (n c)").rearrange(
                        "(p f) -> p f", p=P))
                fb = p0.tile([P, FP], bf16, name="fb")
                nc.vector.tensor_copy(out=fb[:], in_=ft[:])
                nc.sync.dma_start(
                    out=featB.rearrange("(p f) -> p f", p=P), in_=fb[:])
            with (tc.tile_pool(name="ptab", bufs=1) as ptab,
                  tc.tile_pool(name="p1", bufs=2) as p1):
                tab1 = ptab.tile([P, CH, 2], bf16, name="tab1")
                tmpx = ptab.tile([P, nwin1, 2], f32, name="tmpx")
                for ch in range(NCH):
                    ne_ch = min(CH, N - ch * CH)
                    nc.sync.dma_start(
                        out=tab1[:, 0:ne_ch, :].rearrange("p n c -> p (n c)"),
                        in_=featB[ch * 2 * CH:ch * 2 * CH + 2 * ne_ch][None, :]
                        .to_broadcast([P, 2 * ne_ch]))
                    for r in range(R):
                        S1 = meta["L1"][r][ch]["S"]
                        seg = meta["L1"][r][ch]["seg"]
                        go1 = p1.tile([P, S1, 2], bf16,
                                      name=f"go1_{r}_{ch}", tag="go1")
                        ii = p1.tile([P, S1 // 16], i16,
                                     name=f"ii_{r}_{ch}", tag="ii")
                        nc.sync.dma_start(out=ii[:],
                                          in_=ins[f"l1idx_{r}_{ch}"][:])
                        ce = p1.tile([P, S1], bf16,
                                     name=f"ce_{r}_{ch}", tag="ce")
                        nc.sync.dma_start(out=ce[:],
                                          in_=ins[f"l1ew_{r}_{ch}"][:])
                        co = p1.tile([P, S1], bf16,
                                     name=f"co_{r}_{ch}", tag="co")
                        nc.sync.dma_start(out=co[:],
                                          in_=ins[f"l1os_{r}_{ch}"][:])
                        nc.vector.tensor_tensor(out=ce[:], in0=ce[:],
                                                in1=co[:], op=AL.mult)
                        nc.gpsimd.ap_gather(
                            out_ap=go1[:, :, :], in_ap=tab1[:, 0:ne_ch, :],
                            idxs_ap=ii[:, :], channels=P, num_elems=ne_ch,
                            d=2, num_idxs=S1)
                        nc.vector.tensor_tensor(
                            out=go1[:, :, :], in0=go1[:, :, :],
                            in1=ce[:, :, None].to_broadcast([P, S1, 2]),
                            op=AL.mult)
                        reduce_windows(go1, tmpx, seg, nwin1)
                        if ch == 0:
                            nc.vector.tensor_copy(
                                out=x_t[:, :, 2 * r:2 * r + 2], in_=tmpx[:])
                        else:
                            nc.vector.tensor_add(
                                out=x_t[:, :, 2 * r:2 * r + 2],
                                in0=x_t[:, :, 2 * r:2 * r + 2], in1=tmpx[:])
                for r in range(R):
                    il_t = p1.tile([P, nwin1], f32, name=f"il_{r}", tag="il")
                    nc.sync.dma_start(out=il_t[:], in_=ins[f"idsl_{r}"][:])
                    nc.vector.tensor_tensor(
                        out=x_t[:, :, 2 * r:2 * r + 2],
                        in0=x_t[:, :, 2 * r:2 * r + 2],
                        in1=il_t[:, :, None].to_broadcast([P, nwin1, 2]),
                        op=AL.mult)

            # ---- phase 2: h1 = relu(x@W1+b1), g tables (bf16 rows)
            with tc.tile_pool(name="p2", bufs=1) as p2:
                w1_sb = gpool.tile([P, 16 * 2 * R], f32, name="w1_sb")
                nc.sync.dma_start(out=w1_sb[:], in_=w1p[:, :])
                b1all = gpool.tile([P, R * 16], f32, name="b1all")
                nc.sync.dma_start(out=b1all[:], in_=b1b[:, :])
                b1s = gpool.tile([P, 16], f32, name="b1s")
                nc.vector.tensor_reduce(
                    out=b1s[:], in_=b1all[:].rearrange("p (r f) -> p f r", r=R),
                    op=AL.add, axis=mybir.AxisListType.X)
                h1_t = p2.tile([P, nwin1, 16], f32, name="h1_t")
                tmpV = p2.tile([P, nwin1, 16], f32, name="tmpV")
                tmpG = p2.tile([P, nwin1, 16], f32, name="tmpG")
                CR = 2 * R
                for f in range(16):
                    eng, tmp = ((nc.vector, tmpV) if f % 2 == 0
                                else (nc.gpsimd, tmpG))
                    w_ap = w1_sb[:, f * CR:(f + 1) * CR][:, None, :] \
                        .to_broadcast([P, nwin1, CR])
                    eng.tensor_tensor(out=tmp[:, :, 0:CR], in0=x_t[:, :, :],
                                      in1=w_ap, op=AL.mult)
                    nc.vector.tensor_reduce(
                        out=h1_t[:, :, f:f + 1], in_=tmp[:, :, 0:CR],
                        op=AL.add, axis=mybir.AxisListType.X)
                b_ap = b1s[:, None, :].to_broadcast([P, nwin1, 16])
                nc.vector.tensor_tensor(out=h1_t[:, :, :], in0=h1_t[:, :, :],
                                        in1=b_ap, op=AL.add)
                nc.vector.tensor_scalar_max(h1_t[:, :, :], h1_t[:, :, :], 0.0)

                wc_sb = gpool.tile([16, 2], f32, name="wc_sb")
                nc.sync.dma_start(out=wc_sb[:], in_=Wc[:, :])
                m_sb = gpool.tile([1, R * 32], f32, name="m_sb")
                ones_sb = gpool.tile([1, P], f32, name="ones_sb")
                nc.vector.memset(ones_sb[:], 1.0)
                for r in range(R):
                    w2_sb = gpool.tile([16, 16], f32, name=f"w2_{r}", tag="w2")
                    nc.sync.dma_start(out=w2_sb[:],
                                      in_=W2[r, :, :].rearrange("a b -> b a"))
                    m_ps = psum.tile([16, 2], f32, name=f"mps_{r}", tag="mps")
                    nc.tensor.matmul(out=m_ps[:], lhsT=w2_sb[:], rhs=wc_sb[:],
                                     start=True, stop=True)
                    mt = gpool.tile([16, 2], f32, name=f"mt_{r}", tag="mt")
                    nc.vector.tensor_copy(out=mt[:], in_=m_ps[:])
                    md = nc.dram_tensor(f"m_dram_{r}", [16, 2], f32,
                                        kind="Internal").ap()
                    nc.sync.dma_start(out=md[:, :], in_=mt[:])
                    nc.sync.dma_start(out=m_sb[:, r * 32:(r + 1) * 32],
                                      in_=md.rearrange("f c -> (f c)")[None, :])
                mb_ps = psum.tile([P, R * 32], f32, name="mb_ps")
                nc.tensor.matmul(out=mb_ps[:], lhsT=ones_sb[:], rhs=m_sb[:],
                                 start=True, stop=True)
                mb = gpool.tile([P, R * 32], f32, name="mb")
                nc.vector.tensor_copy(out=mb[:], in_=mb_ps[:])
                for r in range(R):
                    g_t = p2.tile([P, nwin1, 2], f32, name=f"g_{r}", tag="g")
                    for cch in range(2):
                        j = 2 * r + cch
                        tmp = tmpV if j % 2 == 0 else tmpG
                        w_ap = mb[:, r * 32:(r + 1) * 32] \
                            .rearrange("p (f c) -> p c f", c=2) \
                            [:, cch:cch + 1, :].to_broadcast([P, nwin1, 16])
                        eng = nc.vector if j % 2 == 0 else nc.gpsimd
                        eng.tensor_tensor(out=tmp[:, :, :], in0=h1_t[:, :, :],
                                          in1=w_ap, op=AL.mult)
                        nc.vector.tensor_reduce(
                            out=g_t[:, :, cch:cch + 1], in_=tmp[:, :, :],
                            op=AL.add, axis=mybir.AxisListType.X)
                    ol_t = p2.tile([P, nwin1], f32, name=f"ol_{r}", tag="ol")
                    nc.sync.dma_start(out=ol_t[:], in_=ins[f"odsl_{r}"][:])
                    g_b = p2.tile([P, nwin1, 2], bf16, name=f"gb_{r}",
                                  tag="gb")
                    nc.vector.tensor_tensor(
                        out=g_b[:, :, :], in0=g_t[:, :, :],
                        in1=ol_t[:, :, None].to_broadcast([P, nwin1, 2]),
                        op=AL.mult)
                    nc.sync.dma_start(
                        out=gtabB[r].rearrange("(p f) -> p f", p=P),
                        in_=g_b[:].rearrange("p k c -> p (k c)"))

            # ---- phase 3: L2 via ap_gather per relation
            with tc.tile_pool(name="p3", bufs=1) as p3:
                S2max = max(meta["L2"][r]["S"] for r in range(R))
                tab2 = p3.tile([P, NE1, 2], bf16, name="tab2")
                go2 = p3.tile([P, S2max, 2], bf16, name="go2")
                for r in range(R):
                    S2 = meta["L2"][r]["S"]
                    seg = meta["L2"][r]["seg"]
                    nc.sync.dma_start(
                        out=tab2[:].rearrange("p n c -> p (n c)"),
                        in_=gtabB[r][None, :].to_broadcast([P, NE1 * 2]))
                    li = p3.tile([P, S2 // 16], i16, name=f"li_{r}", tag="li")
                    nc.sync.dma_start(out=li[:], in_=ins[f"l2idx_{r}"][:])
                    lco = p3.tile([P, S2], bf16, name=f"lc_{r}", tag="lc")
                    nc.sync.dma_start(out=lco[:], in_=ins[f"l2coef_{r}"][:])
                    nc.gpsimd.ap_gather(
                        out_ap=go2[:, 0:S2, :], in_ap=tab2[:, :, :],
                        idxs_ap=li[:, :], channels=P, num_elems=NE1,
                        d=2, num_idxs=S2)
                    nc.vector.tensor_tensor(
                        out=go2[:, 0:S2, :], in0=go2[:, 0:S2, :],
                        in1=lco[:, :, None].to_broadcast([P, S2, 2]),
                        op=AL.mult)
                    pr_t = gpool.tile([P, nwin2, 2], f32, name=f"pr_{r}")
                    j0 = 0
                    for w in range(nwin2):
                        lw = int(seg[w])
                        nc.vector.tensor_reduce(
                            out=pr_t[:, w, :],
                            in_=go2[:, j0:j0 + lw, :].rearrange(
                                "p l c -> p c l"),
                            op=AL.add, axis=mybir.AxisListType.X)
                        j0 += lw
                    nc.sync.dma_start(
                        out=prank[r][:nwin2 * P, :].rearrange(
                            "(p k) c -> p k c", p=P),
                        in_=pr_t[:, :, :])
                    pr_ts.append(pr_t)

            # ---- phase 4: realign graphs (per-column), sum, bias, out
            with tc.tile_pool(name="p4", bufs=2) as p4:
                osum = gpool.tile([P, nwin2, 2], f32, name="osum")
                for r in range(R):
                    gp_t = p4.tile([P, nwin2], i32, name=f"gp_{r}", tag="gp")
                    nc.sync.dma_start(out=gp_t[:], in_=ins[f"gpos_{r}"][:])
                    gr = p4.tile([P, nwin2, 2], f32, name=f"gr_{r}", tag="gr")
                    for c0 in range(nwin2):
                        nc.gpsimd.indirect_dma_start(
                            out=gr[:, c0, :], out_offset=None, in_=prank[r][:],
                            in_offset=bass.IndirectOffsetOnAxis(
                                ap=gp_t[:, c0:c0 + 1], axis=0))
                    if r == 0:
                        nc.vector.tensor_copy(out=osum[:, :, :], in_=gr[:, :, :])
                    else:
                        nc.vector.tensor_add(out=osum[:, :, :],
                                             in0=osum[:, :, :], in1=gr[:, :, :])
                nc.sync.dma_start(
                    out=out_part.rearrange("(k p) c -> p k c", p=P),
                    in_=osum[:, :, :])
                b2all = p4.tile([P, R * 16], f32, name="b2all")
                nc.sync.dma_start(out=b2all[:], in_=b2b[:, :])
                b2s = p4.tile([P, 16], f32, name="b2s")
                nc.vector.tensor_reduce(
                    out=b2s[:], in_=b2all[:].rearrange("p (r f) -> p f r", r=R),
                    op=AL.add, axis=mybir.AxisListType.X)
                b2d = nc.dram_tensor("b2s_dram", [16], f32, kind="Internal").ap()
                nc.sync.dma_start(out=b2d[None, :], in_=b2s[0:1, :])
                b2col = p4.tile([16, 1], f32, name="b2col")
                nc.sync.dma_start(out=b2col[:], in_=b2d[:, None])
                bo_ps = psum.tile([1, 2], f32, name="bo_ps")
                wc2 = p4.tile([16, 2], f32, name="wc2")
                nc.sync.dma_start(out=wc2[:], in_=Wc[:, :])
                nc.tensor.matmul(out=bo_ps[:], lhsT=b2col[:], rhs=wc2[:],
                                 start=True, stop=True)
                bc_sb = p4.tile([1, 2], f32, name="bc_sb")
                nc.sync.dma_start(out=bc_sb[:], in_=bc[None, :])
                bo_sb = p4.tile([1, 2], f32, name="bo_sb")
                nc.vector.tensor_add(out=bo_sb[:], in0=bo_ps[:], in1=bc_sb[:])
                nc.sync.dma_start(out=bias_out[:, :], in_=bo_sb[:])
    nc.compile()
    return nc


def kernel(feat, src, dst, ew, graph_ids, W1, b1, W2, b2, Wc, bc):
    per_core, meta = _prep(feat, src, dst, ew, graph_ids)
    nc = _build_program(meta)
    w1f = np.ascontiguousarray(W1, dtype=np.float32) \
        .transpose(2, 0, 1).reshape(-1)  # [f, (r, c)] f-major for phase-2 FMA
    b1f = np.ascontiguousarray(b1, dtype=np.float32).reshape(-1)
    b2f = np.ascontiguousarray(b2, dtype=np.float32).reshape(-1)
    common = {
        "feat": np.ascontiguousarray(feat, dtype=np.float32),
        "w1p": np.tile(w1f, (P, 1)),
        "b1b": np.tile(b1f, (P, 1)),
        "W2": np.ascontiguousarray(W2, dtype=np.float32),
        "b2b": np.tile(b2f, (P, 1)),
        "Wc": np.ascontiguousarray(Wc, dtype=np.float32),
        "bc": np.ascontiguousarray(bc, dtype=np.float32),
    }
    in_maps = [{**common, **per_core[c]} for c in range(NCORES)]
    import os as _os
    import time as _t
    _t0 = _t.perf_counter()
    res = bass_utils.run_bass_kernel_spmd(
        nc, in_maps, core_ids=list(range(NCORES)),
        tmpdir=_os.environ.get("K_TRACE_DIR") or None)
    global LAST_EXEC_NS, LAST_TRACE
    LAST_EXEC_NS = int((_t.perf_counter() - _t0) * 1e9)
    if res.exec_time_ns:
        LAST_EXEC_NS = int(res.exec_time_ns)
    LAST_TRACE = res.instructions_and_trace[1] if res.instructions_and_trace else None
    out = np.zeros((B, 2), dtype=np.float32)
    for c in range(NCORES):
        out += res.results[c]["out_part"]
    out += res.results[0]["bias_out"][0]
    return out


# revision 43
# speedup vs baseline: 3.8989x; 3.8989x over previous
"""HeteroClassifier GNN kernel for 8 TRN2 NeuronCores (Bass/Tile).

Sharding: L1 edges by dst node-range (owner core aggregates its nodes);
L2 edges by src node-range (gather tables stay core-local); per-core
[B,2] partial outputs are summed to unshard. Host does structure-only
prep (degree counts, edge grouping, padded window/stream layouts, index
maps). All value compute runs on the NeuronCores.

Gather mechanisms (HW-validated):
- L1 and L2 edge gathers: gpsimd ap_gather from SBUF-replicated bf16
  tables (feat in 28672-node chunks for int16 indices; g per relation);
  each 16-partition group processes its own edge stream with masked
  coefficient tables, then windowed segment reduction. Streams are
  double-buffered so gathers overlap the DVE mask/reduce work, and
  reductions land directly in natural (lane, window) order (no realign).
- Final graph realign: per-column indirect DMA (32 columns).
"""

import numpy as np

import concourse.bass as bass
import concourse.bacc as bacc
import concourse.mybir as mybir
import concourse.tile as tile
from concourse import bass_utils

LAST_EXEC_NS = -1
LAST_TRACE = None
N = 200000
R = 4
E = 1000000
B = 1024
NCORES = 8
P = 128
G16 = 16          # partitions per gpsimd group
NG = P // G16     # 8 groups


def _sizes():
    nb = N // NCORES
    nwin1 = (nb + P - 1) // P
    nwin2 = (B + P - 1) // P
    return nb, nwin1, nwin2


def _pack_apg(lane, win, row, coefs, nwin, seg_len):
    """Pack an edge stream for ap_gather with group-shared indices.

    lane/win: target slot (lane in [0,P), win in [0,nwin)); row: int16
    table row to gather; coef: f32 coefficient (applied at target lane).
    seg_len[w]: shared per-window segment length (max over groups/cores).
    Returns (idx_tile [P, S//16] i16, coeff [P, S] f32) with
    S = 16*ceil(sum(seg_len)/16)."""
    S0 = int(np.sum(seg_len))
    S = ((S0 + 15) // 16) * 16
    sbase = np.concatenate([[0], np.cumsum(seg_len)]).astype(np.int64)
    g = lane // G16
    c16 = lane % G16
    # position within (group, window)
    order = np.lexsort((np.arange(len(lane)), win, g))
    gs, ws = g[order], win[order]
    # j_in_seg: running index within each (g, w) bucket
    key = gs * nwin + ws
    ksorted = key  # already sorted by (g, w)
    starts = np.searchsorted(ksorted, np.arange(NG * nwin))
    j_in = np.arange(len(ksorted)) - starts[ksorted]
    jpos = sbase[ws] + j_in
    import ml_dtypes
    idx_tile = np.zeros((P, S // 16), dtype=np.int16)
    rs, ls_ = row[order], c16[order]
    idx_tile[gs * G16 + (jpos % 16), jpos // 16] = rs
    outs = []
    for coef in coefs:
        coeff = np.zeros((P, S), dtype=ml_dtypes.bfloat16)
        coeff[gs * G16 + ls_, jpos] = coef[order]
        outs.append(coeff)
    return idx_tile, outs, S


def _prep(feat, src, dst, ew, graph_ids):
    nb, nwin1, nwin2 = _sizes()
    src = np.asarray(src); dst = np.asarray(dst)
    ew = np.asarray(ew); gid = np.asarray(graph_ids)

    od = np.stack([np.bincount(src[r], minlength=N) for r in range(R)])
    idg = np.stack([np.bincount(dst[r], minlength=N) for r in range(R)])
    cnt = np.bincount(gid, minlength=B)
    ods = (1.0 / np.sqrt(np.clip(od, 1, None))).astype(np.float32)
    ids = (1.0 / np.sqrt(np.clip(idg, 1, None))).astype(np.float32)
    qn = (ids / np.clip(cnt, 1, None)[gid][None, :]).astype(np.float32)

    meta = {"L1": [], "L2": [], "X": {}}
    per_core = [dict() for _ in range(NCORES)]

    # ---- L1: dst-sharded ap_gather streams; gather table = feat in
    # 32768-row bf16 chunks (int16 index limit). Reduce output lands in
    # natural (lane, win) order, so no realign stage is needed.
    CH = 28672
    NCH = (N + CH - 1) // CH
    meta["CH"], meta["NCH"] = CH, NCH
    for r in range(R):
        core_of = dst[r] // nb
        chunk_of = src[r] // CH
        rel_meta = []
        for ch in range(NCH):
            seg = np.ones(nwin1, dtype=np.int64)
            dat = []
            for c in range(NCORES):
                m = (core_of == c) & (chunk_of == ch)
                dl = dst[r][m] - c * nb
                lane = (dl % P).astype(np.int64)
                win = (dl // P).astype(np.int64)
                row = (src[r][m] - ch * CH).astype(np.int64)
                dat.append((lane, win, row, ew[r][m],
                            ods[r][src[r][m]].astype(np.float32)))
                cnts = np.bincount((lane // G16) * nwin1 + win,
                                   minlength=NG * nwin1)
                seg = np.maximum(seg, cnts.reshape(NG, nwin1).max(axis=0))
            # round to multiples of 4 so equal-length windows merge into
            # few strided reduce instructions
            seg = ((seg + 3) // 4) * 4
            S = int(((seg.sum() + 15) // 16) * 16)
            rel_meta.append({"seg": seg.tolist(), "S": S})
            for c in range(NCORES):
                lane, win, row, vew, vos = dat[c]
                idx_t, (cew, cos), S2 = _pack_apg(
                    lane, win, row, [vew, vos], nwin1, seg)
                assert S2 == S
                per_core[c][f"l1idx_{r}_{ch}"] = idx_t
                per_core[c][f"l1ew_{r}_{ch}"] = cew
                per_core[c][f"l1os_{r}_{ch}"] = cos
        meta["L1"].append(rel_meta)
        for c in range(NCORES):
            lo = c * nb
            on = np.zeros(nwin1 * P, dtype=np.float32)
            on[:nb] = ids[r, lo:lo + nb]
            per_core[c][f"idsl_{r}"] = on.reshape(nwin1, P).T.copy()
            on2 = np.zeros(nwin1 * P, dtype=np.float32)
            on2[:nb] = ods[r, lo:lo + nb]
            per_core[c][f"odsl_{r}"] = on2.reshape(nwin1, P).T.copy()

    # ---- L2: src-sharded, ap_gather streams grouped by graph rank
    # shared graph ranking per (relation): by per-core counts is fine but
    # ranks must be shared across cores? gpos handles per-core; use global
    # per-relation ranking by total count so window segments are shared.
    for r in range(R):
        gcnt = np.bincount(gid[dst[r]], minlength=B)
        gorder = np.argsort(-gcnt, kind="stable")
        grp = np.empty(B, dtype=np.int64)
        grp[gorder] = np.arange(B)
        core_of = src[r] // nb
        # shared segment lengths: max over cores/groups per window
        seg = np.zeros(nwin2, dtype=np.int64)
        percore_dat = []
        for c in range(NCORES):
            m = core_of == c
            d = dst[r][m]
            rk = grp[gid[d]]
            lane = rk % P
            win = rk // P
            n = src[r][m] - c * nb
            row = (n % P) * nwin1 + n // P
            coefv = qn[r][d]
            percore_dat.append((lane, win, row, coefv))
            cnts = np.bincount((lane // G16) * nwin2 + win,
                               minlength=NG * nwin2)
            seg = np.maximum(seg, cnts.reshape(NG, nwin2).max(axis=0))
        S2 = int(((seg.sum() + 15) // 16) * 16)
        meta["L2"].append({"seg": seg.tolist(), "S": S2})
        for c in range(NCORES):
            lane, win, row, coefv = percore_dat[c]
            idx_t, (coeff,), S = _pack_apg(lane, win, row.astype(np.int64),
                                           [coefv], nwin2, seg)
            assert S == S2
            per_core[c][f"l2idx_{r}"] = idx_t
            per_core[c][f"l2coef_{r}"] = coeff
        # gpos: natural graph b=(k*P+p) -> rank row (lane-major) in prank
        for c in range(NCORES):
            pos = np.full(nwin2 * P, nwin2 * P, dtype=np.int32)
            bb = np.arange(B)
            rk = grp[bb]
            pos[bb] = ((rk % P) * nwin2 + rk // P).astype(np.int32)
            per_core[c][f"gpos_{r}"] = pos.reshape(nwin2, P).T.copy()

    return per_core, meta


def _build_program(meta):
    nb, nwin1, nwin2 = _sizes()
    nc = bacc.Bacc("TRN2", target_bir_lowering=False, debug=False,
                   num_devices=NCORES)
    f32, i32 = mybir.dt.float32, mybir.dt.int32
    bf16, i16 = mybir.dt.bfloat16, mybir.dt.int16
    AL = mybir.AluOpType

    feat = nc.dram_tensor("feat", [N, 2], f32, kind="ExternalInput").ap()
    w1p = nc.dram_tensor("w1p", [P, 16 * 2 * R], f32, kind="ExternalInput").ap()
    b1b = nc.dram_tensor("b1b", [P, R * 16], f32, kind="ExternalInput").ap()
    W2 = nc.dram_tensor("W2", [R, 16, 16], f32, kind="ExternalInput").ap()
    b2b = nc.dram_tensor("b2b", [P, R * 16], f32, kind="ExternalInput").ap()
    Wc = nc.dram_tensor("Wc", [16, 2], f32, kind="ExternalInput").ap()
    bc = nc.dram_tensor("bc", [2], f32, kind="ExternalInput").ap()
    CH, NCH = meta["CH"], meta["NCH"]
    ins = {}
    for r in range(R):
        S2 = meta["L2"][r]["S"]
        for nm, shp, dt in (
            (f"odsl_{r}", [P, nwin1], f32), (f"idsl_{r}", [P, nwin1], f32),
            (f"l2idx_{r}", [P, S2 // 16], i16), (f"l2coef_{r}", [P, S2], bf16),
            (f"gpos_{r}", [P, nwin2], i32),
        ):
            ins[nm] = nc.dram_tensor(nm, shp, dt, kind="ExternalInput").ap()
        for ch in range(NCH):
            S1 = meta["L1"][r][ch]["S"]
            for nm, shp, dt in (
                (f"l1idx_{r}_{ch}", [P, S1 // 16], i16),
                (f"l1ew_{r}_{ch}", [P, S1], bf16),
                (f"l1os_{r}_{ch}", [P, S1], bf16),
            ):
                ins[nm] = nc.dram_tensor(nm, shp, dt, kind="ExternalInput").ap()
    featB = nc.dram_tensor("featB", [N * 2], bf16, kind="Internal").ap()
    gtabB = [nc.dram_tensor(f"gB_{r}", [nwin1 * P * 2], bf16, kind="Internal").ap()
             for r in range(R)]
    prank = [nc.dram_tensor(f"prank_{r}", [(nwin2 + 1) * P, 2], f32, kind="Internal").ap()
             for r in range(R)]
    out_part = nc.dram_tensor("out_part", [B, 2], f32, kind="ExternalOutput").ap()
    bias_out = nc.dram_tensor("bias_out", [1, 2], f32, kind="ExternalOutput").ap()

    NE1 = nwin1 * P  # 25088 table rows

    def reduce_windows(ga, out_t, ls, nwin):
        col = 0
        k = 0
        while k < nwin:
            k2 = k
            while k2 < nwin and ls[k2] == ls[k]:
                k2 += 1
            lk, nk = ls[k], k2 - k
            seg = ga[:, col:col + nk * lk, :].rearrange(
                "p (n l) c -> p n c l", l=lk)
            nc.vector.tensor_reduce(out=out_t[:, k:k2, :], in_=seg,
                                    op=AL.add, axis=mybir.AxisListType.X)
            col += nk * lk
            k = k2

    with tile.TileContext(nc) as tc:
        with (tc.tile_pool(name="glob", bufs=1) as gpool,
              tc.tile_pool(name="psum", bufs=2, space="PSUM") as psum):
            zt = gpool.tile([P, 2], f32, name="zt")
            nc.vector.memset(zt[:], 0.0)
            for r in range(R):
                nc.sync.dma_start(out=prank[r][nwin2 * P:, :], in_=zt[:])

            pr_ts = []

            # ---- phase 1: L1 via chunked ap_gather from bf16 feat
            # tables; masked coefficients; reduce lands in natural order
            x_t = gpool.tile([P, nwin1, 2 * R], f32, name="x_t")
            with tc.tile_pool(name="p0", bufs=1) as p0:
                FP = (N * 2) // P
                ft = p0.tile([P, FP], f32, name="ft")
                nc.sync.dma_start(
                    out=ft[:],
                    in_=feat.rearrange("n c -> (n c)").rearrange(
                        "(p f) -> p f", p=P))
                fb = p0.tile([P, FP], bf16, name="fb")
                nc.vector.tensor_copy(out=fb[:], in_=ft[:])
                nc.sync.dma_start(
                    out=featB.rearrange("(p f) -> p f", p=P), in_=fb[:])
            with (tc.tile_pool(name="ptab", bufs=1) as ptab,
                  tc.tile_pool(name="p1", bufs=2) as p1):
                tab1 = ptab.tile([P, CH, 2], bf16, name="tab1")
                tmpx = ptab.tile([P, nwin1, 2], f32, name="tmpx")
                for ch in range(NCH):
                    ne_ch = min(CH, N - ch * CH)
                    nc.sync.dma_start(
                        out=tab1[:, 0:ne_ch, :].rearrange("p n c -> p (n c)"),
                        in_=featB[ch * 2 * CH:ch * 2 * CH + 2 * ne_ch][None, :]
                        .to_broadcast([P, 2 * ne_ch]))
                    for r in range(R):
                        S1 = meta["L1"][r][ch]["S"]
                        seg = meta["L1"][r][ch]["seg"]
                        go1 = p1.tile([P, S1, 2], bf16,
                                      name=f"go1_{r}_{ch}", tag="go1")
                        ii = p1.tile([P, S1 // 16], i16,
                                     name=f"ii_{r}_{ch}", tag="ii")
                        nc.sync.dma_start(out=ii[:],
                                          in_=ins[f"l1idx_{r}_{ch}"][:])
                        ce = p1.tile([P, S1], bf16,
                                     name=f"ce_{r}_{ch}", tag="ce")
                        nc.sync.dma_start(out=ce[:],
                                          in_=ins[f"l1ew_{r}_{ch}"][:])
                        co = p1.tile([P, S1], bf16,
                                     name=f"co_{r}_{ch}", tag="co")
                        nc.sync.dma_start(out=co[:],
                                          in_=ins[f"l1os_{r}_{ch}"][:])
                        nc.vector.tensor_tensor(out=ce[:], in0=ce[:],
                                                in1=co[:], op=AL.mult)
                        nc.gpsimd.ap_gather(
                            out_ap=go1[:, :, :], in_ap=tab1[:, 0:ne_ch, :],
                            idxs_ap=ii[:, :], channels=P, num_elems=ne_ch,
                            d=2, num_idxs=S1)
                        nc.vector.tensor_tensor(
                            out=go1[:, :, :], in0=go1[:, :, :],
                            in1=ce[:, :, None].to_broadcast([P, S1, 2]),
                            op=AL.mult)
                        reduce_windows(go1, tmpx, seg, nwin1)
                        if ch == 0:
                            nc.vector.tensor_copy(
                                out=x_t[:, :, 2 * r:2 * r + 2], in_=tmpx[:])
                        else:
                            nc.vector.tensor_add(
                                out=x_t[:, :, 2 * r:2 * r + 2],
                                in0=x_t[:, :, 2 * r:2 * r + 2], in1=tmpx[:])
                for r in range(R):
                    il_t = p1.tile([P, nwin1], f32, name=f"il_{r}", tag="il")
                    nc.sync.dma_start(out=il_t[:], in_=ins[f"idsl_{r}"][:])
                    nc.vector.tensor_tensor(
                        out=x_t[:, :, 2 * r:2 * r + 2],
                        in0=x_t[:, :, 2 * r:2 * r + 2],
                        in1=il_t[:, :, None].to_broadcast([P, nwin1, 2]),
                        op=AL.mult)

            # ---- phase 2: h1 = relu(x@W1+b1), g tables (bf16 rows)
            with tc.tile_pool(name="p2", bufs=1) as p2:
                w1_sb = gpool.tile([P, 16 * 2 * R], f32, name="w1_sb")
                nc.sync.dma_start(out=w1_sb[:], in_=w1p[:, :])
                b1all = gpool.tile([P, R * 16], f32, name="b1all")
                nc.sync.dma_start(out=b1all[:], in_=b1b[:, :])
                b1s = gpool.tile([P, 16], f32, name="b1s")
                nc.vector.tensor_reduce(
                    out=b1s[:], in_=b1all[:].rearrange("p (r f) -> p f r", r=R),
                    op=AL.add, axis=mybir.AxisListType.X)
                h1_t = p2.tile([P, nwin1, 16], f32, name="h1_t")
                tmpV = p2.tile([P, nwin1, 16], f32, name="tmpV")
                tmpG = p2.tile([P, nwin1, 16], f32, name="tmpG")
                CR = 2 * R
                for f in range(16):
                    eng, tmp = ((nc.vector, tmpV) if f % 2 == 0
                                else (nc.gpsimd, tmpG))
                    w_ap = w1_sb[:, f * CR:(f + 1) * CR][:, None, :] \
                        .to_broadcast([P, nwin1, CR])
                    eng.tensor_tensor(out=tmp[:, :, 0:CR], in0=x_t[:, :, :],
                                      in1=w_ap, op=AL.mult)
                    nc.vector.tensor_reduce(
                        out=h1_t[:, :, f:f + 1], in_=tmp[:, :, 0:CR],
                        op=AL.add, axis=mybir.AxisListType.X)
                b_ap = b1s[:, None, :].to_broadcast([P, nwin1, 16])
                nc.vector.tensor_tensor(out=h1_t[:, :, :], in0=h1_t[:, :, :],
                                        in1=b_ap, op=AL.add)
                nc.vector.tensor_scalar_max(h1_t[:, :, :], h1_t[:, :, :], 0.0)

                wc_sb = gpool.tile([16, 2], f32, name="wc_sb")
                nc.sync.dma_start(out=wc_sb[:], in_=Wc[:, :])
                m_sb = gpool.tile([1, R * 32], f32, name="m_sb")
                ones_sb = gpool.tile([1, P], f32, name="ones_sb")
                nc.vector.memset(ones_sb[:], 1.0)
                for r in range(R):
                    w2_sb = gpool.tile([16, 16], f32, name=f"w2_{r}", tag="w2")
                    nc.sync.dma_start(out=w2_sb[:],
                                      in_=W2[r, :, :].rearrange("a b -> b a"))
                    m_ps = psum.tile([16, 2], f32, name=f"mps_{r}", tag="mps")
                    nc.tensor.matmul(out=m_ps[:], lhsT=w2_sb[:], rhs=wc_sb[:],
                                     start=True, stop=True)
                    mt = gpool.tile([16, 2], f32, name=f"mt_{r}", tag="mt")
                    nc.vector.tensor_copy(out=mt[:], in_=m_ps[:])
                    md = nc.dram_tensor(f"m_dram_{r}", [16, 2], f32,
                                        kind="Internal").ap()
                    nc.sync.dma_start(out=md[:, :], in_=mt[:])
                    nc.sync.dma_start(out=m_sb[:, r * 32:(r + 1) * 32],
                                      in_=md.rearrange("f c -> (f c)")[None, :])
                mb_ps = psum.tile([P, R * 32], f32, name="mb_ps")
                nc.tensor.matmul(out=mb_ps[:], lhsT=ones_sb[:], rhs=m_sb[:],
                                 start=True, stop=True)
                mb = gpool.tile([P, R * 32], f32, name="mb")
                nc.vector.tensor_copy(out=mb[:], in_=mb_ps[:])
                for r in range(R):
                    g_t = p2.tile([P, nwin1, 2], f32, name=f"g_{r}", tag="g")
                    for cch in range(2):
                        j = 2 * r + cch
                        tmp = tmpV if j % 2 == 0 else tmpG
                        w_ap = mb[:, r * 32:(r + 1) * 32] \
                            .rearrange("p (f c) -> p c f", c=2) \
                            [:, cch:cch + 1, :].to_broadcast([P, nwin1, 16])
                        eng = nc.vector if j % 2 == 0 else nc.gpsimd
                        eng.tensor_tensor(out=tmp[:, :, :], in0=h1_t[:, :, :],
                                          in1=w_ap, op=AL.mult)
                        nc.vector.tensor_reduce(
                            out=g_t[:, :, cch:cch + 1], in_=tmp[:, :, :],
                            op=AL.add, axis=mybir.AxisListType.X)
                    ol_t = p2.tile([P, nwin1], f32, name=f"ol_{r}", tag="ol")
                    nc.sync.dma_start(out=ol_t[:], in_=ins[f"odsl_{r}"][:])
                    g_b = p2.tile([P, nwin1, 2], bf16, name=f"gb_{r}",
                                  tag="gb")
                    nc.vector.tensor_tensor(
                        out=g_b[:, :, :], in0=g_t[:, :, :],
                        in1=ol_t[:, :, None].to_broadcast([P, nwin1, 2]),
                        op=AL.mult)
                    nc.sync.dma_start(
                        out=gtabB[r].rearrange("(p f) -> p f", p=P),
                        in_=g_b[:].rearrange("p k c -> p (k c)"))

            # ---- phase 3: L2 via ap_gather per relation
            with tc.tile_pool(name="p3", bufs=1) as p3:
                S2max = max(meta["L2"][r]["S"] for r in range(R))
                tab2 = p3.tile([P, NE1, 2], bf16, name="tab2")
                go2 = p3.tile([P, S2max, 2], bf16, name="go2")
                for r in range(R):
                    S2 = meta["L2"][r]["S"]
                    seg = meta["L2"][r]["seg"]
                    nc.sync.dma_start(
                        out=tab2[:].rearrange("p n c -> p (n c)"),
                        in_=gtabB[r][None, :].to_broadcast([P, NE1 * 2]))
                    li = p3.tile([P, S2 // 16], i16, name=f"li_{r}", tag="li")
                    nc.sync.dma_start(out=li[:], in_=ins[f"l2idx_{r}"][:])
                    lco = p3.tile([P, S2], bf16, name=f"lc_{r}", tag="lc")
                    nc.sync.dma_start(out=lco[:], in_=ins[f"l2coef_{r}"][:])
                    nc.gpsimd.ap_gather(
                        out_ap=go2[:, 0:S2, :], in_ap=tab2[:, :, :],
                        idxs_ap=li[:, :], channels=P, num_elems=NE1,
                        d=2, num_idxs=S2)
                    nc.vector.tensor_tensor(
                        out=go2[:, 0:S2, :], in0=go2[:, 0:S2, :],
                        in1=lco[:, :, None].to_broadcast([P, S2, 2]),
                        op=AL.mult)
                    pr_t = gpool.tile([P, nwin2, 2], f32, name=f"pr_{r}")
                    j0 = 0
                    for w in range(nwin2):
                        lw = int(seg[w])
                        nc.vector.tensor_reduce(
                            out=pr_t[:, w, :],
                            in_=go2[:, j0:j0 + lw, :].rearrange(
                                "p l c -> p c l"),
                            op=AL.add, axis=mybir.AxisListType.X)
                        j0 += lw
                    nc.sync.dma_start(
                        out=prank[r][:nwin2 * P, :].rearrange(
                            "(p k) c -> p k c", p=P),
                        in_=pr_t[:, :, :])
                    pr_ts.append(pr_t)

            # ---- phase 4: realign graphs (per-column), sum, bias, out
            with tc.tile_pool(name="p4", bufs=2) as p4:
                osum = gpool.tile([P, nwin2, 2], f32, name="osum")
                for r in range(R):
                    gp_t = p4.tile([P, nwin2], i32, name=f"gp_{r}", tag="gp")
                    nc.sync.dma_start(out=gp_t[:], in_=ins[f"gpos_{r}"][:])
                    gr = p4.tile([P, nwin2, 2], f32, name=f"gr_{r}", tag="gr")
                    for c0 in range(nwin2):
                        nc.gpsimd.indirect_dma_start(
                            out=gr[:, c0, :], out_offset=None, in_=prank[r][:],
                            in_offset=bass.IndirectOffsetOnAxis(
                                ap=gp_t[:, c0:c0 + 1], axis=0))
                    if r == 0:
                        nc.vector.tensor_copy(out=osum[:, :, :], in_=gr[:, :, :])
                    else:
                        nc.vector.tensor_add(out=osum[:, :, :],
                                             in0=osum[:, :, :], in1=gr[:, :, :])
                nc.sync.dma_start(
                    out=out_part.rearrange("(k p) c -> p k c", p=P),
                    in_=osum[:, :, :])
                b2all = p4.tile([P, R * 16], f32, name="b2all")
                nc.sync.dma_start(out=b2all[:], in_=b2b[:, :])
                b2s = p4.tile([P, 16], f32, name="b2s")
                nc.vector.tensor_reduce(
                    out=b2s[:], in_=b2all[:].rearrange("p (r f) -> p f r", r=R),
                    op=AL.add, axis=mybir.AxisListType.X)
                b2d = nc.dram_tensor("b2s_dram", [16], f32, kind="Internal").ap()
                nc.sync.dma_start(out=b2d[None, :], in_=b2s[0:1, :])
                b2col = p4.tile([16, 1], f32, name="b2col")
                nc.sync.dma_start(out=b2col[:], in_=b2d[:, None])
                bo_ps = psum.tile([1, 2], f32, name="bo_ps")
                wc2 = p4.tile([16, 2], f32, name="wc2")
                nc.sync.dma_start(out=wc2[:], in_=Wc[:, :])
                nc.tensor.matmul(out=bo_ps[:], lhsT=b2col[:], rhs=wc2[:],
                                 start=True, stop=True)
                bc_sb = p4.tile([1, 2], f32, name="bc_sb")
                nc.sync.dma_start(out=bc_sb[:], in_=bc[None, :])
                bo_sb = p4.tile([1, 2], f32, name="bo_sb")
                nc.vector.tensor_add(out=bo_sb[:], in0=bo_ps[:], in1=bc_sb[:])
                nc.sync.dma_start(out=bias_out[:, :], in_=bo_sb[:])
    nc.compile()
    return nc


def kernel(feat, src, dst, ew, graph_ids, W1, b1, W2, b2, Wc, bc):
    per_core, meta = _prep(feat, src, dst, ew, graph_ids)
    nc = _build_program(meta)
    w1f = np.ascontiguousarray(W1, dtype=np.float32) \
        .transpose(2, 0, 1).reshape(-1)  # [f, (r, c)] f-major for phase-2 FMA
    b1f = np.ascontiguousarray(b1, dtype=np.float32).reshape(-1)
    b2f = np.ascontiguousarray(b2, dtype=np.float32).reshape(-1)
    common = {
        "feat": np.ascontiguousarray(feat, dtype=np.float32),
        "w1p": np.tile(w1f, (P, 1)),
        "b1b": np.tile(b1f, (P, 1)),
        "W2": np.ascontiguousarray(W2, dtype=np.float32),
        "b2b": np.tile(b2f, (P, 1)),
        "Wc": np.ascontiguousarray(Wc, dtype=np.float32),
        "bc": np.ascontiguousarray(bc, dtype=np.float32),
    }
    in_maps = [{**common, **per_core[c]} for c in range(NCORES)]
    import os as _os
    import time as _t
    _t0 = _t.perf_counter()
    res = bass_utils.run_bass_kernel_spmd(
        nc, in_maps, core_ids=list(range(NCORES)),
        tmpdir=_os.environ.get("K_TRACE_DIR") or None)
    global LAST_EXEC_NS, LAST_TRACE
    LAST_EXEC_NS = int((_t.perf_counter() - _t0) * 1e9)
    if res.exec_time_ns:
        LAST_EXEC_NS = int(res.exec_time_ns)
    LAST_TRACE = res.instructions_and_trace[1] if res.instructions_and_trace else None
    out = np.zeros((B, 2), dtype=np.float32)
    for c in range(NCORES):
        out += res.results[c]["out_part"]
    out += res.results[0]["bias_out"][0]
    return out
